# revision 31
# baseline (speedup 1.0000x reference)
"""Trainium2 Bass kernel for batch-axis-softmax dot-product attention.

Problem: B=8, S=4096, D=64 fp32.
    scores = einsum('bqd,bkd->bqk', Q, K) / 8
    attn   = softmax(scores, axis=0)          # over the BATCH axis!
    out    = einsum('bqk,bkd->bqd', attn, V)

The batch-axis softmax couples only the 8 batch entries of a fixed (q, k)
position, so sharding over the *query* axis (512 queries per core, K/V
replicated) keeps the softmax fully local to each core.

Per-core pipeline, per super-tile T (two 128-key k-tiles, 512 queries, 8
batches):
  PE : scoresT[k,q] = K_tile @ Q^T per batch pair (fp16, fp32 psum, row-
       tiled 64-deep matmul pairs; 2-bank psum packs, ping-pong)
  ACT: exp packs -> e_all[T] [128, 8192] fp16, column order (t, j, m, q).
       ScalarE does ONLY exp: 4 ops/k-tile of FD 1024 at (1024+352)cyc/1.2GHz
       is the kernel's ~147us floor (PSUM has no room for bigger score
       tiles: oacc needs 4 of the 8 banks, the score ping-pong the rest);
       1/Z lives on DVE instead (the ln+exp R-pass cost ~37us of ScalarE
       in the previous version).
  DVE: 5 ops per super-tile (vs 18 before; each ~88-cycle fixed overhead),
       all within ~3% of the measured per-op formula floor:
       a = e01 + e23 pairs (one 3D-AP add, FD 4096, fp16 2x)
       s = a-halves summed (FD 2048)
       Z = s-halves summed (FD 1024) = [Z(t0) | Z(t1)] fp16
       R = RECIPROCAL_APPROX_FAST(Z) -- single custom-DVE op, fp16 in/out
       (validated on HW: ~5e-4 rel err; DVE converts fp16->fp32 on ingest
       so the BITWISE_NOT fp32-bit-trick seed still works)
       W_all = e_all * R-broadcast (one FD-8192 tensor_tensor in 2x mode)
  PE : outT_b[d,q] += V_tile matmul per batch, accumulated over all 32
       k-tiles in persistent psum (2 batches per bank via column tiling)
Epilogue: ScalarE copies psum -> sbuf as fp16, one flat DMA to HBM (one
4KB descriptor per partition; a j-transposed dst AP cost ~11.6us here),
host casts/reassembles.

Scheduling: per-super-tile software pipeline. The DVE backend for T is
emitted after the front end of T+1; AV matmuls of T are staged and only
released into the between-score-packs drain queue during the front end
of T+3 -- released earlier they reach the PE queue head before the
producing mult finishes and head-of-line-block the score matmuls that
feed ScalarE. First/last backends are split per k-tile (earlier DVE
start / shorter tail). Steady state is ACT-paced at ~9.2us per super-
tile with DVE ~97% busy; both engines sit at their structural floors
for this dataflow (ScalarE 1 elem/cycle/lane exp; DVE 2x-mode port
bandwidth on the tree + normalize mult).
"""

import numpy as np

B = 8
S = 4096
D = 64
NCORES = 8
QBLK = S // NCORES  # 512 queries per core
KT = 128            # keys per k-tile
NKT = S // KT       # 32 k-tiles
NPAIR = B // 2      # batch pairs packed into 128 partitions
NST = NKT // 2      # 16 super-tiles (2 k-tiles each)

# test.py can flip these before calling kernel()
TRACE = False
TRACE_KWARGS = {}
LAST_RESULT = None  # BassKernelResults of the most recent run (for profiling)

_cache = {}


def _build_nc():
    from contextlib import ExitStack

    import concourse.tile as tile
    from concourse import bacc, mybir
    from concourse.dve_ops import RECIP_APPROX_FAST_CONSTS, RECIPROCAL_APPROX_FAST

    f16 = mybir.dt.float16
    f32 = mybir.dt.float32
    Exp = mybir.ActivationFunctionType.Exp

    nc = bacc.Bacc()

    # Inputs pre-arranged on host into exact SBUF layouts (fp16):
    #   qt[p, j*512 + q] = Q[2j + p//64, cblk*512 + q, p%64]
    #   kt[p, j*4096 + k] = K[2j + p//64, k, p%64]
    #   vv[p, b*2048 + n*64 + d] = V[b, n*128 + p, d]
    qt_d = nc.dram_tensor("qt", [128, NPAIR * QBLK], f16, kind="ExternalInput")
    kt_d = nc.dram_tensor("kt", [128, NPAIR * S], f16, kind="ExternalInput")
    vv_d = nc.dram_tensor("vv", [128, B * NKT * D], f16, kind="ExternalInput")
    # out[(b%2)*64 + d, j*512 + q] = out_bqd[2j + b%2, q, d]; fp16 (|out| ~
    # O(1), well within fp16 and the 2e-2 rel-err budget) -- halves the final
    # DMA -- and FLAT per-partition (one 4KB descriptor per partition instead
    # of 512x 1KB ones; the j-transposed layout made the tail DMA ~11.6us).
    out_d = nc.dram_tensor("out", [128, NPAIR * QBLK], f16, kind="ExternalOutput")

    with tile.TileContext(nc) as tc, ExitStack() as ctx:
        in_p = ctx.enter_context(tc.tile_pool(name="inp", bufs=1))
        e_p = ctx.enter_context(tc.tile_pool(name="e", bufs=3))
        w_p = ctx.enter_context(tc.tile_pool(name="w", bufs=3))
        # a/s/z/r are written and read only by DVE, whose queue is in-order:
        # a single buffer per tag needs no cross-engine WAR tracking slack.
        t_p = ctx.enter_context(tc.tile_pool(name="tree", bufs=1))
        r_p = ctx.enter_context(tc.tile_pool(name="r", bufs=1))
        st_p = ctx.enter_context(tc.tile_pool(name="stage", bufs=1))
        ps_s = ctx.enter_context(tc.tile_pool(name="ps_s", bufs=2, space="PSUM"))
        ps_o = ctx.enter_context(tc.tile_pool(name="ps_o", bufs=1, space="PSUM"))

        # kt/vv are laid out k-tile-major on the host and DMA'd in per-tile
        # chunks interleaved kt/vv, so tile 0's operands land ~7us in and the
        # loop never waits on later chunks.
        qt = in_p.tile([128, NPAIR * QBLK], f16)
        kt = in_p.tile([128, NKT * NPAIR * KT], f16)
        vv = in_p.tile([128, NKT * B * D], f16)
        CH = NPAIR * KT  # 512 columns per k-tile chunk (for both kt and vv)

        def dma_col(dst, src, c0, c1):
            nc.sync.dma_start(out=dst[:, c0:c1], in_=src[:, c0:c1])

        # Issue order: operands of score pack (t=0, j=0) first (the j0 slice
        # of kt chunk 0 on its own so the first matmul starts ASAP), then the
        # rest of tile 0, then per-tile chunks so the loop never waits. vv
        # chunks are not needed until the AV matmuls (~3 super-tiles in), so
        # they trail the kt chunks slightly.
        dma_col(kt, kt_d, 0, KT)
        dma_col(qt, qt_d, 0, QBLK)
        dma_col(kt, kt_d, KT, CH)
        for j in range(1, NPAIR):
            dma_col(qt, qt_d, j * QBLK, (j + 1) * QBLK)
        dma_col(kt, kt_d, CH, 2 * CH)
        dma_col(vv, vv_d, 0, CH)
        for t in range(2, NKT):
            dma_col(kt, kt_d, t * CH, (t + 1) * CH)
            dma_col(vv, vv_d, (t - 1) * CH, t * CH)
        dma_col(vv, vv_d, (NKT - 1) * CH, NKT * CH)

        # Persistent output accumulators: bank j holds batches 2j (parts
        # 0:64) and 2j+1 (parts 64:128), accumulated over all 32 k-tiles.
        oacc = [
            ps_o.tile([128, QBLK], f32, tag=f"oacc{j}", name=f"oacc{j}")
            for j in range(NPAIR)
        ]

        # AV matmuls pending issue; drained between score packs so PE always
        # services the (ACT-feeding) score matmuls promptly instead of
        # running long AV bursts that starve ScalarE. Interleaving AV MMs
        # of adjacent k-tiles is safe: psum accumulate-adds commute.
        av_pending = []

        def drain_av(n):
            for _ in range(min(n, len(av_pending))):
                av_pending.pop(0)()

        def emit_scores_exp(T, tr, e_all):
            # scores + exp for k-tile t = 2T + tr; exp writes into the
            # (t, j)-slice of e_all so the whole super-tile is contiguous.
            t = 2 * T + tr
            for j in range(NPAIR):
                sc = ps_s.tile([128, 2 * QBLK], f32, tag="sc", name=f"sc{t}_{j}")
                for m in range(2):  # m=0 -> b=2j (rows 0:64), m=1 -> b=2j+1
                    rb = m * 64
                    nc.tensor.matmul(
                        out=sc[:, m * QBLK : (m + 1) * QBLK],
                        lhsT=kt[rb : rb + 64, t * CH + j * KT : t * CH + (j + 1) * KT],
                        rhs=qt[rb : rb + 64, j * QBLK : (j + 1) * QBLK],
                        start=True,
                        stop=True,
                        tile_position=(rb, 0),
                    )
                # E = exp(scores / sqrt(D)); scores*0.125 in [-6, 6] so no
                # max-subtraction is needed and fp16 range is safe.
                c0 = tr * 4096 + j * 1024
                nc.scalar.activation(e_all[:, c0 : c0 + 1024], sc[:], Exp, scale=0.125)
                drain_av(2)

        def backend_part(T, e_all, w, trs):
            # Z = sum over the 8 batches, R = 1/Z, W = E*R: five DVE ops
            # covering k-tiles 2T+tr for tr in trs (the whole super-tile in
            # steady state; per-k-tile halves for the last one to shrink the
            # kernel tail). e_all column order is (t, j, m, q); adds and the
            # mult are fp16 2x mode; R is one 1x custom op.
            nt = len(trs)
            t0 = trs[0]
            ec = e_all[:, t0 * 4096 : (t0 + nt) * 4096]
            a = t_p.tile([128, 4096], f16, tag="a", name=f"a{T}_{t0}")
            ev = ec.rearrange("p (t g x) -> p t g x", t=nt, g=2)
            nc.vector.tensor_add(
                a[:, : nt * 2048].rearrange("p (t g x) -> p t g x", t=nt, g=2),
                ev[:, :, :, 0:1024],
                ev[:, :, :, 1024:2048],
            )
            s = t_p.tile([128, 2048], f16, tag="s", name=f"s{T}_{t0}")
            av_ = a[:, : nt * 2048].rearrange("p (t g x) -> p t g x", t=nt, g=2)
            nc.vector.tensor_add(
                s[:, : nt * 1024].rearrange("p (t x) -> p t x", t=nt),
                av_[:, :, 0, :],
                av_[:, :, 1, :],
            )
            z = r_p.tile([128, 1024], f16, tag="z", name=f"z{T}_{t0}")
            sv = s[:, : nt * 1024].rearrange("p (t h x) -> p t h x", t=nt, h=2)
            nc.vector.tensor_add(
                z[:, : nt * 512].rearrange("p (t x) -> p t x", t=nt),
                sv[:, :, 0, :],
                sv[:, :, 1, :],
            )
            # R = 1/Z in one custom-DVE op (bitwise-NOT seed + 2 inline NR
            # passes); fp16 in/out is fine: DVE converts to fp32 on ingest.
            r = r_p.tile([128, 1024], f16, tag="r", name=f"r{T}_{t0}")
            cc = RECIP_APPROX_FAST_CONSTS
            nc.vector._custom_dve(
                RECIPROCAL_APPROX_FAST,
                out=r[:, : nt * 512],
                in0=z[:, : nt * 512],
                s0=cc["s0"],
                s1=cc["s1"],
                imm2=cc["imm2"],
            )
            # W = E * R, one fp16 2x op with R broadcast over (j, m) via a
            # stride-0 access pattern.
            nc.vector.tensor_mul(
                w[:, t0 * 4096 : (t0 + nt) * 4096].rearrange(
                    "p (t a x) -> p t a x", t=nt, a=8
                ),
                ec.rearrange("p (t a x) -> p t a x", t=nt, a=8),
                r[:, : nt * 512]
                .rearrange("p (t a x) -> p t a x", t=nt, a=1)
                .to_broadcast((128, nt, 8, QBLK)),
            )
            avs = []
            for tr in trs:
                avs.extend(emit_av(T, tr, w))
            return avs

        def emit_av(T, tr, w):
            # outT_b[d,q] += V_b[t]^T-form matmul, queued for interleaved
            # issue (see drain_av). Reverse order so the first-issued AV's
            # wait (on the mult's DVE tick) covers the others.
            t = 2 * T + tr

            def mk(b):
                j, m = b // 2, b % 2
                rb = m * 64
                c0 = tr * 4096 + j * 1024 + m * QBLK

                def go():
                    nc.tensor.matmul(
                        out=oacc[j][rb : rb + 64, :],
                        lhsT=vv[:, t * CH + b * D : t * CH + (b + 1) * D],
                        rhs=w[:, c0 : c0 + QBLK],
                        start=(t == 0),
                        stop=(t == NKT - 1),
                        tile_position=(0, rb),
                        skip_group_check=True,
                    )

                return go

            return [mk(b) for b in reversed(range(B))]

        # Software pipeline: the DVE backend for super-tile T is emitted
        # after the front end (scores+exp) of super-tile T+1, so every
        # cross-engine input reaching an engine's strict FIFO was produced
        # a super-tile earlier and waits are pre-satisfied. AV matmuls of T
        # are STAGED and only released into the drain queue during the front
        # end of T+3: releasing at T+2 put them at the PE queue head before
        # the producing mult finished, head-of-line-blocking the score
        # matmuls and stalling ScalarE ~1.5us per super-tile. The last
        # super-tile's backend runs per k-tile so its first half overlaps
        # the final exps (shorter kernel tail).
        e_tiles = {}
        av_stage = {}
        for T in range(NST + 1):
            if T - 3 in av_stage:
                av_pending.extend(av_stage.pop(T - 3))
            if T < NST:
                e_all = e_p.tile([128, 8192], f16, tag="e", name=f"e{T}")
                e_tiles[T] = e_all
                emit_scores_exp(T, 0, e_all)
                emit_scores_exp(T, 1, e_all)
            if T == 0:
                # Split the first backend per k-tile (its first half needs
                # only 4 exps -> DVE's pipeline starts ~4us earlier).
                w = w_p.tile([128, 8192], f16, tag="w", name="w0")
                av_stage[0] = backend_part(0, e_tiles[0], w, [0])
                av_stage[0] += backend_part(0, e_tiles.pop(0), w, [1])
            if T >= 2:
                X = T - 1
                w = w_p.tile([128, 8192], f16, tag="w", name=f"w{X}")
                if X == NST - 1:
                    # Split the last backend too: halves the mult->AV->copy
                    # latency chain in the kernel tail.
                    av_stage[X] = backend_part(X, e_tiles[X], w, [0])
                    av_stage[X] += backend_part(X, e_tiles.pop(X), w, [1])
                else:
                    av_stage[X] = backend_part(X, e_tiles.pop(X), w, [0, 1])
        for X in sorted(av_stage):
            av_pending.extend(av_stage.pop(X))
        drain_av(len(av_pending))

        # Epilogue: psum -> sbuf copies on ScalarE (idle in the tail, and
        # its PSUM read port is the fast one) as fp16 -- halves the final
        # DMA -- then one output DMA (4 separate dma_starts each pay ~2us
        # of setup/completion latency in the kernel tail).
        st = st_p.tile([128, NPAIR * QBLK], f16, tag="st")
        for j in range(NPAIR):
            nc.scalar.copy(out=st[:, j * QBLK : (j + 1) * QBLK], in_=oacc[j][:])
        nc.sync.dma_start(out=out_d[:], in_=st[:])

    return nc


def _get_nc():
    if "nc" not in _cache:
        nc = _build_nc()
        if not nc.is_finalized():
            # Runs Bacc.compile() legalization (wait splitting, reg alloc).
            nc.finalize()
        _cache["nc"] = nc
    return _cache["nc"]


def _host_prep(queries, keys, values):
    """Cast to fp16 and pre-arrange into the SBUF layouts (see _build_nc)."""
    k16 = np.asarray(keys, dtype=np.float16)
    v16 = np.asarray(values, dtype=np.float16)
    q16 = np.asarray(queries, dtype=np.float16)

    # kt[(b%2)*64+d, t*512 + (b//2)*128 + kk] = K[b, t*128+kk, d] (k-tile major)
    kt = np.ascontiguousarray(
        k16.reshape(NPAIR, 2, NKT, KT, D)
        .transpose(1, 4, 2, 0, 3)
        .reshape(128, NKT * NPAIR * KT)
    )
    # vv[p, t*512 + b*64 + d] = V[b, t*128+p, d] (k-tile major)
    vv = np.ascontiguousarray(
        v16.reshape(B, NKT, KT, D).transpose(2, 1, 0, 3).reshape(128, NKT * B * D)
    )

    qts = []
    for c in range(NCORES):
        qc = q16[:, c * QBLK : (c + 1) * QBLK, :]  # [8, 512, 64]
        qt = np.ascontiguousarray(
            qc.transpose(0, 2, 1).reshape(NPAIR, 128, QBLK).transpose(1, 0, 2).reshape(128, NPAIR * QBLK)
        )
        qts.append(qt)
    return qts, kt, vv


def kernel(queries, keys, values):
    global LAST_RESULT
    from concourse.bass_utils import run_bass_kernel_spmd

    queries = np.asarray(queries, dtype=np.float32)
    keys = np.asarray(keys, dtype=np.float32)
    values = np.asarray(values, dtype=np.float32)

    nc = _get_nc()
    qts, kt, vv = _host_prep(queries, keys, values)
    in_maps = [{"qt": qts[c], "kt": kt, "vv": vv} for c in range(NCORES)]

    res = run_bass_kernel_spmd(
        nc,
        in_maps,
        list(range(NCORES)),
        trace=TRACE,
        **TRACE_KWARGS,
    )
    LAST_RESULT = res

    out = np.empty((B, S, D), dtype=np.float32)
    for c in range(NCORES):
        o = res.results[c]["out"]  # [128, 2048] fp16 = [(b%2)*64+d, j*512+q]
        out[:, c * QBLK : (c + 1) * QBLK, :] = (
            o.astype(np.float32)
            .reshape(2, D, NPAIR, QBLK)
            .transpose(2, 0, 3, 1)
            .reshape(B, QBLK, D)
        )
    return out


# revision 32
# speedup vs baseline: 1.1926x; 1.1926x over previous
"""Trainium2 Bass kernel for batch-axis-softmax dot-product attention.

Problem: B=8, S=4096, D=64 fp32.
    scores = einsum('bqd,bkd->bqk', Q, K) / 8
    attn   = softmax(scores, axis=0)          # over the BATCH axis!
    out    = einsum('bqk,bkd->bqd', attn, V)

The batch-axis softmax couples only the 8 batch entries of a fixed (q, k)
position, so sharding over the *query* axis (512 queries per core, K/V
replicated) keeps the softmax fully local to each core.

Per-core pipeline, per super-tile T (two 128-key k-tiles, 512 queries, 8
batches):
  PE : scoresT[k,q] = K_tile @ Q^T per batch pair (fp16, fp32 psum, row-
       tiled 64-deep matmul pairs; 2-bank psum packs, ping-pong)
  ACT: exp packs -> e_all[T] [128, 8192] fp16, column order (t, j, m, q).
       ScalarE does ONLY exp: 4 ops/k-tile of FD 1024 at (1024+352)cyc/1.2GHz
       is the kernel's ~147us floor (PSUM has no room for bigger score
       tiles: oacc needs 4 of the 8 banks, the score ping-pong the rest);
       1/Z lives on DVE instead (the ln+exp R-pass cost ~37us of ScalarE
       in the previous version).
  DVE: 5 ops per super-tile (vs 18 before; each ~88-cycle fixed overhead),
       all within ~3% of the measured per-op formula floor:
       a = e01 + e23 pairs (one 3D-AP add, FD 4096, fp16 2x)
       s = a-halves summed (FD 2048)
       Z = s-halves summed (FD 1024) = [Z(t0) | Z(t1)] fp16
       R = RECIPROCAL_APPROX_FAST(Z) -- single custom-DVE op, fp16 in/out
       (validated on HW: ~5e-4 rel err; DVE converts fp16->fp32 on ingest
       so the BITWISE_NOT fp32-bit-trick seed still works)
       W_all = e_all * R-broadcast (one FD-8192 tensor_tensor in 2x mode)
  PE : outT_b[d,q] += V_tile matmul per batch, accumulated over all 32
       k-tiles in persistent psum (2 batches per bank via column tiling)
Epilogue: ScalarE copies psum -> sbuf as fp16, one flat DMA to HBM (one
4KB descriptor per partition; a j-transposed dst AP cost ~11.6us here),
host casts/reassembles.

Scheduling: per-super-tile software pipeline. The DVE backend for T is
emitted after the front end of T+1; AV matmuls of T are staged and only
released into the between-score-packs drain queue during the front end
of T+3 -- released earlier they reach the PE queue head before the
producing mult finishes and head-of-line-block the score matmuls that
feed ScalarE. First/last backends are split per k-tile (earlier DVE
start / shorter tail). Steady state is ACT-paced at ~9.2us per super-
tile with DVE ~97% busy; both engines sit at their structural floors
for this dataflow (ScalarE 1 elem/cycle/lane exp; DVE 2x-mode port
bandwidth on the tree + normalize mult).
"""

import numpy as np

B = 8
S = 4096
D = 64
NCORES = 8
QBLK = S // NCORES  # 512 queries per core
KT = 128            # keys per k-tile
NKT = S // KT       # 32 k-tiles
NPAIR = B // 2      # batch pairs packed into 128 partitions
NST = NKT // 2      # 16 super-tiles (2 k-tiles each)

# test.py can flip these before calling kernel()
TRACE = False
TRACE_KWARGS = {}
LAST_RESULT = None  # BassKernelResults of the most recent run (for profiling)

_cache = {}


def _build_nc():
    from contextlib import ExitStack

    import concourse.tile as tile
    from concourse import bacc, mybir
    from concourse.dve_ops import RECIP_APPROX_FAST_CONSTS, RECIPROCAL_APPROX_FAST

    f16 = mybir.dt.float16
    f32 = mybir.dt.float32
    Exp = mybir.ActivationFunctionType.Exp

    nc = bacc.Bacc()

    # Inputs pre-arranged on host into exact SBUF layouts (fp16):
    #   qt[p, j*512 + q] = Q[2j + p//64, cblk*512 + q, p%64]
    #   kt[p, j*4096 + k] = K[2j + p//64, k, p%64]
    #   vv[p, b*2048 + n*64 + d] = V[b, n*128 + p, d]
    qt_d = nc.dram_tensor("qt", [128, NPAIR * QBLK], f16, kind="ExternalInput")
    kt_d = nc.dram_tensor("kt", [128, NPAIR * S], f16, kind="ExternalInput")
    vv_d = nc.dram_tensor("vv", [128, B * NKT * D], f16, kind="ExternalInput")
    # out[(b%2)*64 + d, j*512 + q] = out_bqd[2j + b%2, q, d]; fp16 (|out| ~
    # O(1), well within fp16 and the 2e-2 rel-err budget) -- halves the final
    # DMA -- and FLAT per-partition (one 4KB descriptor per partition instead
    # of 512x 1KB ones; the j-transposed layout made the tail DMA ~11.6us).
    out_d = nc.dram_tensor("out", [128, NPAIR * QBLK], f16, kind="ExternalOutput")

    with tile.TileContext(nc) as tc, ExitStack() as ctx:
        in_p = ctx.enter_context(tc.tile_pool(name="inp", bufs=1))
        e_p = ctx.enter_context(tc.tile_pool(name="e", bufs=4))
        w_p = ctx.enter_context(tc.tile_pool(name="w", bufs=3))
        # a/s/z/r are written and read only by DVE, whose queue is in-order:
        # a single buffer per tag needs no cross-engine WAR tracking slack.
        t_p = ctx.enter_context(tc.tile_pool(name="tree", bufs=1))
        r_p = ctx.enter_context(tc.tile_pool(name="r", bufs=1))
        st_p = ctx.enter_context(tc.tile_pool(name="stage", bufs=1))
        ps_s = ctx.enter_context(tc.tile_pool(name="ps_s", bufs=2, space="PSUM"))
        ps_o = ctx.enter_context(tc.tile_pool(name="ps_o", bufs=1, space="PSUM"))

        # kt/vv are laid out k-tile-major on the host and DMA'd in per-tile
        # chunks interleaved kt/vv, so tile 0's operands land ~7us in and the
        # loop never waits on later chunks.
        qt = in_p.tile([128, NPAIR * QBLK], f16)
        kt = in_p.tile([128, NKT * NPAIR * KT], f16)
        vv = in_p.tile([128, NKT * B * D], f16)
        CH = NPAIR * KT  # 512 columns per k-tile chunk (for both kt and vv)

        def dma_col(dst, src, c0, c1):
            nc.sync.dma_start(out=dst[:, c0:c1], in_=src[:, c0:c1])

        # Issue order: operands of score pack (t=0, j=0) first (the j0 slice
        # of kt chunk 0 on its own so the first matmul starts ASAP), then the
        # rest of tile 0, then per-tile chunks so the loop never waits. vv
        # chunks are not needed until the AV matmuls (~3 super-tiles in), so
        # they trail the kt chunks slightly.
        dma_col(kt, kt_d, 0, KT)
        dma_col(qt, qt_d, 0, QBLK)
        dma_col(kt, kt_d, KT, CH)
        for j in range(1, NPAIR):
            dma_col(qt, qt_d, j * QBLK, (j + 1) * QBLK)
        dma_col(kt, kt_d, CH, 2 * CH)
        dma_col(vv, vv_d, 0, CH)
        for t in range(2, NKT):
            dma_col(kt, kt_d, t * CH, (t + 1) * CH)
            dma_col(vv, vv_d, (t - 1) * CH, t * CH)
        dma_col(vv, vv_d, (NKT - 1) * CH, NKT * CH)

        # Persistent output accumulators: bank j holds batches 2j (parts
        # 0:64) and 2j+1 (parts 64:128), accumulated over all 32 k-tiles.
        oacc = [
            ps_o.tile([128, QBLK], f32, tag=f"oacc{j}", name=f"oacc{j}")
            for j in range(NPAIR)
        ]

        # AV matmuls pending issue; drained between score packs so PE always
        # services the (ACT-feeding) score matmuls promptly instead of
        # running long AV bursts that starve ScalarE. Interleaving AV MMs
        # of adjacent k-tiles is safe: psum accumulate-adds commute.
        av_pending = []

        def drain_av(n):
            for _ in range(min(n, len(av_pending))):
                av_pending.pop(0)()

        def emit_scores_exp(T, tr, e_all):
            # scores + exp for k-tile t = 2T + tr; exp writes into the
            # (t, j)-slice of e_all so the whole super-tile is contiguous.
            t = 2 * T + tr
            for j in range(NPAIR):
                sc = ps_s.tile([128, 2 * QBLK], f32, tag="sc", name=f"sc{t}_{j}")
                for m in range(2):  # m=0 -> b=2j (rows 0:64), m=1 -> b=2j+1
                    rb = m * 64
                    nc.tensor.matmul(
                        out=sc[:, m * QBLK : (m + 1) * QBLK],
                        lhsT=kt[rb : rb + 64, t * CH + j * KT : t * CH + (j + 1) * KT],
                        rhs=qt[rb : rb + 64, j * QBLK : (j + 1) * QBLK],
                        start=True,
                        stop=True,
                        tile_position=(rb, 0),
                    )
                # E = exp(scores / sqrt(D)); scores*0.125 in [-6, 6] so no
                # max-subtraction is needed and fp16 range is safe.
                c0 = tr * 4096 + j * 1024
                nc.scalar.activation(e_all[:, c0 : c0 + 1024], sc[:], Exp, scale=0.125)
                drain_av(2)

        def backend_part(T, e_all, w, trs):
            # Z = sum over the 8 batches, R = 1/Z, W = E*R: five DVE ops
            # covering k-tiles 2T+tr for tr in trs (the whole super-tile in
            # steady state; per-k-tile halves for the last one to shrink the
            # kernel tail). e_all column order is (t, j, m, q); adds and the
            # mult are fp16 2x mode; R is one 1x custom op.
            nt = len(trs)
            t0 = trs[0]
            ec = e_all[:, t0 * 4096 : (t0 + nt) * 4096]
            a = t_p.tile([128, 4096], f16, tag="a", name=f"a{T}_{t0}")
            ev = ec.rearrange("p (t g x) -> p t g x", t=nt, g=2)
            nc.vector.tensor_add(
                a[:, : nt * 2048].rearrange("p (t g x) -> p t g x", t=nt, g=2),
                ev[:, :, :, 0:1024],
                ev[:, :, :, 1024:2048],
            )
            s = t_p.tile([128, 2048], f16, tag="s", name=f"s{T}_{t0}")
            av_ = a[:, : nt * 2048].rearrange("p (t g x) -> p t g x", t=nt, g=2)
            nc.vector.tensor_add(
                s[:, : nt * 1024].rearrange("p (t x) -> p t x", t=nt),
                av_[:, :, 0, :],
                av_[:, :, 1, :],
            )
            z = r_p.tile([128, 1024], f16, tag="z", name=f"z{T}_{t0}")
            sv = s[:, : nt * 1024].rearrange("p (t h x) -> p t h x", t=nt, h=2)
            nc.vector.tensor_add(
                z[:, : nt * 512].rearrange("p (t x) -> p t x", t=nt),
                sv[:, :, 0, :],
                sv[:, :, 1, :],
            )
            # R = 1/Z in one custom-DVE op (bitwise-NOT seed + 2 inline NR
            # passes); fp16 in/out is fine: DVE converts to fp32 on ingest.
            r = r_p.tile([128, 1024], f16, tag="r", name=f"r{T}_{t0}")
            cc = RECIP_APPROX_FAST_CONSTS
            nc.vector._custom_dve(
                RECIPROCAL_APPROX_FAST,
                out=r[:, : nt * 512],
                in0=z[:, : nt * 512],
                s0=cc["s0"],
                s1=cc["s1"],
                imm2=cc["imm2"],
            )
            # W = E * R, one fp16 2x op with R broadcast over (j, m) via a
            # stride-0 access pattern.
            nc.vector.tensor_mul(
                w[:, t0 * 4096 : (t0 + nt) * 4096].rearrange(
                    "p (t a x) -> p t a x", t=nt, a=8
                ),
                ec.rearrange("p (t a x) -> p t a x", t=nt, a=8),
                r[:, : nt * 512]
                .rearrange("p (t a x) -> p t a x", t=nt, a=1)
                .to_broadcast((128, nt, 8, QBLK)),
            )
            avs = []
            for tr in trs:
                avs.extend(emit_av(T, tr, w))
            return avs

        def emit_av(T, tr, w):
            # outT_b[d,q] += V_b[t]^T-form matmul, queued for interleaved
            # issue (see drain_av). Reverse order so the first-issued AV's
            # wait (on the mult's DVE tick) covers the others.
            t = 2 * T + tr

            def mk(b):
                j, m = b // 2, b % 2
                rb = m * 64
                c0 = tr * 4096 + j * 1024 + m * QBLK

                def go():
                    nc.tensor.matmul(
                        out=oacc[j][rb : rb + 64, :],
                        lhsT=vv[:, t * CH + b * D : t * CH + (b + 1) * D],
                        rhs=w[:, c0 : c0 + QBLK],
                        start=(t == 0),
                        stop=(t == NKT - 1),
                        tile_position=(0, rb),
                        skip_group_check=True,
                    )

                return go

            return [mk(b) for b in reversed(range(B))]

        # Software pipeline: the DVE backend for super-tile T is emitted
        # after the front end (scores+exp) of super-tile T+1, so every
        # cross-engine input reaching an engine's strict FIFO was produced
        # a super-tile earlier and waits are pre-satisfied. AV matmuls of T
        # are STAGED and only released into the drain queue during the front
        # end of T+3: releasing at T+2 put them at the PE queue head before
        # the producing mult finished, head-of-line-blocking the score
        # matmuls and stalling ScalarE ~1.5us per super-tile. The last
        # super-tile's backend runs per k-tile so its first half overlaps
        # the final exps (shorter kernel tail).
        e_tiles = {}
        av_stage = {}
        for T in range(NST + 1):
            if T - 3 in av_stage:
                av_pending.extend(av_stage.pop(T - 3))
            if T < NST:
                e_all = e_p.tile([128, 8192], f16, tag="e", name=f"e{T}")
                e_tiles[T] = e_all
                emit_scores_exp(T, 0, e_all)
                emit_scores_exp(T, 1, e_all)
            if T == 0:
                # Split the first backend per k-tile (its first half needs
                # only 4 exps -> DVE's pipeline starts ~4us earlier).
                w = w_p.tile([128, 8192], f16, tag="w", name="w0")
                av_stage[0] = backend_part(0, e_tiles[0], w, [0])
                av_stage[0] += backend_part(0, e_tiles.pop(0), w, [1])
            if T >= 2:
                X = T - 1
                w = w_p.tile([128, 8192], f16, tag="w", name=f"w{X}")
                if X == NST - 1:
                    # Split the last backend too: halves the mult->AV->copy
                    # latency chain in the kernel tail.
                    av_stage[X] = backend_part(X, e_tiles[X], w, [0])
                    av_stage[X] += backend_part(X, e_tiles.pop(X), w, [1])
                else:
                    av_stage[X] = backend_part(X, e_tiles.pop(X), w, [0, 1])
        for X in sorted(av_stage):
            av_pending.extend(av_stage.pop(X))
        drain_av(len(av_pending))

        # Epilogue: psum -> sbuf copies on ScalarE (idle in the tail, and
        # its PSUM read port is the fast one) as fp16 -- halves the final
        # DMA -- then one output DMA (4 separate dma_starts each pay ~2us
        # of setup/completion latency in the kernel tail).
        st = st_p.tile([128, NPAIR * QBLK], f16, tag="st")
        for j in range(NPAIR):
            nc.scalar.copy(out=st[:, j * QBLK : (j + 1) * QBLK], in_=oacc[j][:])
        nc.sync.dma_start(out=out_d[:], in_=st[:])

    return nc


def _get_nc():
    if "nc" not in _cache:
        nc = _build_nc()
        if not nc.is_finalized():
            # Runs Bacc.compile() legalization (wait splitting, reg alloc).
            nc.finalize()
        _cache["nc"] = nc
    return _cache["nc"]


def _host_prep(queries, keys, values):
    """Cast to fp16 and pre-arrange into the SBUF layouts (see _build_nc)."""
    k16 = np.asarray(keys, dtype=np.float16)
    v16 = np.asarray(values, dtype=np.float16)
    q16 = np.asarray(queries, dtype=np.float16)

    # kt[(b%2)*64+d, t*512 + (b//2)*128 + kk] = K[b, t*128+kk, d] (k-tile major)
    kt = np.ascontiguousarray(
        k16.reshape(NPAIR, 2, NKT, KT, D)
        .transpose(1, 4, 2, 0, 3)
        .reshape(128, NKT * NPAIR * KT)
    )
    # vv[p, t*512 + b*64 + d] = V[b, t*128+p, d] (k-tile major)
    vv = np.ascontiguousarray(
        v16.reshape(B, NKT, KT, D).transpose(2, 1, 0, 3).reshape(128, NKT * B * D)
    )

    qts = []
    for c in range(NCORES):
        qc = q16[:, c * QBLK : (c + 1) * QBLK, :]  # [8, 512, 64]
        qt = np.ascontiguousarray(
            qc.transpose(0, 2, 1).reshape(NPAIR, 128, QBLK).transpose(1, 0, 2).reshape(128, NPAIR * QBLK)
        )
        qts.append(qt)
    return qts, kt, vv


def kernel(queries, keys, values):
    global LAST_RESULT
    from concourse.bass_utils import run_bass_kernel_spmd

    queries = np.asarray(queries, dtype=np.float32)
    keys = np.asarray(keys, dtype=np.float32)
    values = np.asarray(values, dtype=np.float32)

    nc = _get_nc()
    qts, kt, vv = _host_prep(queries, keys, values)
    in_maps = [{"qt": qts[c], "kt": kt, "vv": vv} for c in range(NCORES)]

    res = run_bass_kernel_spmd(
        nc,
        in_maps,
        list(range(NCORES)),
        trace=TRACE,
        **TRACE_KWARGS,
    )
    LAST_RESULT = res

    out = np.empty((B, S, D), dtype=np.float32)
    for c in range(NCORES):
        o = res.results[c]["out"]  # [128, 2048] fp16 = [(b%2)*64+d, j*512+q]
        out[:, c * QBLK : (c + 1) * QBLK, :] = (
            o.astype(np.float32)
            .reshape(2, D, NPAIR, QBLK)
            .transpose(2, 0, 3, 1)
            .reshape(B, QBLK, D)
        )
    return out


# revision 33
# speedup vs baseline: 1.2010x; 1.0071x over previous
"""Trainium2 Bass kernel for batch-axis-softmax dot-product attention.

Problem: B=8, S=4096, D=64 fp32.
    scores = einsum('bqd,bkd->bqk', Q, K) / 8
    attn   = softmax(scores, axis=0)          # over the BATCH axis!
    out    = einsum('bqk,bkd->bqd', attn, V)

The batch-axis softmax couples only the 8 batch entries of a fixed (q, k)
position, so sharding over the *query* axis (512 queries per core, K/V
replicated) keeps the softmax fully local to each core.

Per-core pipeline, per super-tile T (two 128-key k-tiles, 512 queries, 8
batches):
  PE : scoresT[k,q] = K_tile @ Q^T per batch pair (fp16, fp32 psum, row-
       tiled 64-deep matmul pairs; 2-bank psum packs, ping-pong)
  ACT: exp packs -> e_all[T] [128, 8192] fp16, column order (t, j, m, q).
       ScalarE does ONLY exp: 4 ops/k-tile of FD 1024 at (1024+352)cyc/1.2GHz
       is the kernel's ~147us floor (PSUM has no room for bigger score
       tiles: oacc needs 4 of the 8 banks, the score ping-pong the rest);
       1/Z lives on DVE instead (the ln+exp R-pass cost ~37us of ScalarE
       in the previous version).
  DVE: 5 ops per super-tile (vs 18 before; each ~88-cycle fixed overhead),
       all within ~3% of the measured per-op formula floor:
       a = e01 + e23 pairs (one 3D-AP add, FD 4096, fp16 2x)
       s = a-halves summed (FD 2048)
       Z = s-halves summed (FD 1024) = [Z(t0) | Z(t1)] fp16
       R = RECIPROCAL_APPROX_FAST(Z) -- single custom-DVE op, fp16 in/out
       (validated on HW: ~5e-4 rel err; DVE converts fp16->fp32 on ingest
       so the BITWISE_NOT fp32-bit-trick seed still works)
       W_all = e_all * R-broadcast (one FD-8192 tensor_tensor in 2x mode)
  PE : outT_b[d,q] += V_tile matmul per batch, accumulated over all 32
       k-tiles in persistent psum (2 batches per bank via column tiling)
Epilogue: ScalarE copies psum -> sbuf as fp16, one flat DMA to HBM (one
4KB descriptor per partition; a j-transposed dst AP cost ~11.6us here),
host casts/reassembles.

Scheduling: per-super-tile software pipeline. The DVE backend for T is
emitted after the front end of T+1; AV matmuls of T are staged and only
released into the between-score-packs drain queue during the front end
of T+3 -- released earlier they reach the PE queue head before the
producing mult finishes and head-of-line-block the score matmuls that
feed ScalarE. First/last backends are split per k-tile (earlier DVE
start / shorter tail). Steady state is ACT-paced at ~9.2us per super-
tile with DVE ~97% busy; both engines sit at their structural floors
for this dataflow (ScalarE 1 elem/cycle/lane exp; DVE 2x-mode port
bandwidth on the tree + normalize mult).
"""

import numpy as np

B = 8
S = 4096
D = 64
NCORES = 8
QBLK = S // NCORES  # 512 queries per core
KT = 128            # keys per k-tile
NKT = S // KT       # 32 k-tiles
NPAIR = B // 2      # batch pairs packed into 128 partitions
NST = NKT // 2      # 16 super-tiles (2 k-tiles each)

# test.py can flip these before calling kernel()
TRACE = False
TRACE_KWARGS = {}
LAST_RESULT = None  # BassKernelResults of the most recent run (for profiling)

_cache = {}


def _build_nc():
    from contextlib import ExitStack

    import concourse.tile as tile
    from concourse import bacc, mybir
    from concourse.dve_ops import RECIP_APPROX_FAST_CONSTS, RECIPROCAL_APPROX_FAST

    f16 = mybir.dt.float16
    f32 = mybir.dt.float32
    Exp = mybir.ActivationFunctionType.Exp

    nc = bacc.Bacc()

    # Inputs pre-arranged on host into exact SBUF layouts (fp16):
    #   qt[p, j*512 + q] = Q[2j + p//64, cblk*512 + q, p%64]
    #   kt[p, j*4096 + k] = K[2j + p//64, k, p%64]
    #   vv[p, b*2048 + n*64 + d] = V[b, n*128 + p, d]
    qt_d = nc.dram_tensor("qt", [128, NPAIR * QBLK], f16, kind="ExternalInput")
    kt_d = nc.dram_tensor("kt", [128, NPAIR * S], f16, kind="ExternalInput")
    vv_d = nc.dram_tensor("vv", [128, B * NKT * D], f16, kind="ExternalInput")
    # out[(b%2)*64 + d, j*512 + q] = out_bqd[2j + b%2, q, d]; fp16 (|out| ~
    # O(1), well within fp16 and the 2e-2 rel-err budget) -- halves the final
    # DMA -- and FLAT per-partition (one 4KB descriptor per partition instead
    # of 512x 1KB ones; the j-transposed layout made the tail DMA ~11.6us).
    out_d = nc.dram_tensor("out", [128, NPAIR * QBLK], f16, kind="ExternalOutput")

    with tile.TileContext(nc) as tc, ExitStack() as ctx:
        in_p = ctx.enter_context(tc.tile_pool(name="inp", bufs=1))
        e_p = ctx.enter_context(tc.tile_pool(name="e", bufs=4))
        w_p = ctx.enter_context(tc.tile_pool(name="w", bufs=3))
        # a/s/z/r are written and read only by DVE, whose queue is in-order:
        # a single buffer per tag needs no cross-engine WAR tracking slack.
        t_p = ctx.enter_context(tc.tile_pool(name="tree", bufs=1))
        r_p = ctx.enter_context(tc.tile_pool(name="r", bufs=1))
        st_p = ctx.enter_context(tc.tile_pool(name="stage", bufs=1))
        ps_s = ctx.enter_context(tc.tile_pool(name="ps_s", bufs=2, space="PSUM"))
        ps_o = ctx.enter_context(tc.tile_pool(name="ps_o", bufs=1, space="PSUM"))

        # kt/vv are laid out k-tile-major on the host and DMA'd in per-tile
        # chunks interleaved kt/vv, so tile 0's operands land ~7us in and the
        # loop never waits on later chunks.
        qt = in_p.tile([128, NPAIR * QBLK], f16)
        kt = in_p.tile([128, NKT * NPAIR * KT], f16)
        vv = in_p.tile([128, NKT * B * D], f16)
        CH = NPAIR * KT  # 512 columns per k-tile chunk (for both kt and vv)

        def dma_col(dst, src, c0, c1):
            nc.sync.dma_start(out=dst[:, c0:c1], in_=src[:, c0:c1])

        # Issue order: operands of score pack (t=0, j=0) first (the j0 slice
        # of kt chunk 0 on its own so the first matmul starts ASAP), then the
        # rest of tile 0, then per-tile chunks so the loop never waits. vv
        # chunks are not needed until the AV matmuls (~3 super-tiles in), so
        # they trail the kt chunks slightly.
        dma_col(kt, kt_d, 0, KT)
        dma_col(qt, qt_d, 0, QBLK)
        dma_col(kt, kt_d, KT, CH)
        for j in range(1, NPAIR):
            dma_col(qt, qt_d, j * QBLK, (j + 1) * QBLK)
        dma_col(kt, kt_d, CH, 2 * CH)
        dma_col(vv, vv_d, 0, CH)
        for t in range(2, NKT):
            dma_col(kt, kt_d, t * CH, (t + 1) * CH)
            dma_col(vv, vv_d, (t - 1) * CH, t * CH)
        dma_col(vv, vv_d, (NKT - 1) * CH, NKT * CH)

        # Persistent output accumulators: bank j holds batches 2j (parts
        # 0:64) and 2j+1 (parts 64:128), accumulated over all 32 k-tiles.
        oacc = [
            ps_o.tile([128, QBLK], f32, tag=f"oacc{j}", name=f"oacc{j}")
            for j in range(NPAIR)
        ]

        # AV matmuls pending issue; drained between score packs so PE always
        # services the (ACT-feeding) score matmuls promptly instead of
        # running long AV bursts that starve ScalarE. Interleaving AV MMs
        # of adjacent k-tiles is safe: psum accumulate-adds commute.
        av_pending = []

        def drain_av(n):
            for _ in range(min(n, len(av_pending))):
                av_pending.pop(0)()

        def emit_scores_exp(T, tr, e_all):
            # scores + exp for k-tile t = 2T + tr; exp writes into the
            # (t, j)-slice of e_all so the whole super-tile is contiguous.
            t = 2 * T + tr
            for j in range(NPAIR):
                sc = ps_s.tile([128, 2 * QBLK], f32, tag="sc", name=f"sc{t}_{j}")
                for m in range(2):  # m=0 -> b=2j (rows 0:64), m=1 -> b=2j+1
                    rb = m * 64
                    nc.tensor.matmul(
                        out=sc[:, m * QBLK : (m + 1) * QBLK],
                        lhsT=kt[rb : rb + 64, t * CH + j * KT : t * CH + (j + 1) * KT],
                        rhs=qt[rb : rb + 64, j * QBLK : (j + 1) * QBLK],
                        start=True,
                        stop=True,
                        tile_position=(rb, 0),
                    )
                # E = exp(scores / sqrt(D)); scores*0.125 in [-6, 6] so no
                # max-subtraction is needed and fp16 range is safe.
                c0 = tr * 4096 + j * 1024
                nc.scalar.activation(e_all[:, c0 : c0 + 1024], sc[:], Exp, scale=0.125)
                drain_av(2)

        def backend_part(T, e_all, w, trs):
            # Z = sum over the 8 batches, R = 1/Z, W = E*R: five DVE ops
            # covering k-tiles 2T+tr for tr in trs (the whole super-tile in
            # steady state; per-k-tile halves for the last one to shrink the
            # kernel tail). e_all column order is (t, j, m, q); adds and the
            # mult are fp16 2x mode; R is one 1x custom op.
            nt = len(trs)
            t0 = trs[0]
            ec = e_all[:, t0 * 4096 : (t0 + nt) * 4096]
            a = t_p.tile([128, 4096], f16, tag="a", name=f"a{T}_{t0}")
            ev = ec.rearrange("p (t g x) -> p t g x", t=nt, g=2)
            nc.vector.tensor_add(
                a[:, : nt * 2048].rearrange("p (t g x) -> p t g x", t=nt, g=2),
                ev[:, :, :, 0:1024],
                ev[:, :, :, 1024:2048],
            )
            s = t_p.tile([128, 2048], f16, tag="s", name=f"s{T}_{t0}")
            av_ = a[:, : nt * 2048].rearrange("p (t g x) -> p t g x", t=nt, g=2)
            nc.vector.tensor_add(
                s[:, : nt * 1024].rearrange("p (t x) -> p t x", t=nt),
                av_[:, :, 0, :],
                av_[:, :, 1, :],
            )
            z = r_p.tile([128, 1024], f16, tag="z", name=f"z{T}_{t0}")
            sv = s[:, : nt * 1024].rearrange("p (t h x) -> p t h x", t=nt, h=2)
            nc.vector.tensor_add(
                z[:, : nt * 512].rearrange("p (t x) -> p t x", t=nt),
                sv[:, :, 0, :],
                sv[:, :, 1, :],
            )
            # R = 1/Z in one custom-DVE op (bitwise-NOT seed + 2 inline NR
            # passes); fp16 in/out is fine: DVE converts to fp32 on ingest.
            r = r_p.tile([128, 1024], f16, tag="r", name=f"r{T}_{t0}")
            cc = RECIP_APPROX_FAST_CONSTS
            nc.vector._custom_dve(
                RECIPROCAL_APPROX_FAST,
                out=r[:, : nt * 512],
                in0=z[:, : nt * 512],
                s0=cc["s0"],
                s1=cc["s1"],
                imm2=cc["imm2"],
            )
            # W = E * R, one fp16 2x op with R broadcast over (j, m) via a
            # stride-0 access pattern.
            nc.vector.tensor_mul(
                w[:, t0 * 4096 : (t0 + nt) * 4096].rearrange(
                    "p (t a x) -> p t a x", t=nt, a=8
                ),
                ec.rearrange("p (t a x) -> p t a x", t=nt, a=8),
                r[:, : nt * 512]
                .rearrange("p (t a x) -> p t a x", t=nt, a=1)
                .to_broadcast((128, nt, 8, QBLK)),
            )
            avs = []
            for tr in trs:
                avs.extend(emit_av(T, tr, w))
            return avs

        def emit_av(T, tr, w):
            # outT_b[d,q] += V_b[t]^T-form matmul, queued for interleaved
            # issue (see drain_av). Reverse order so the first-issued AV's
            # wait (on the mult's DVE tick) covers the others.
            t = 2 * T + tr

            def mk(b):
                j, m = b // 2, b % 2
                rb = m * 64
                c0 = tr * 4096 + j * 1024 + m * QBLK

                def go():
                    nc.tensor.matmul(
                        out=oacc[j][rb : rb + 64, :],
                        lhsT=vv[:, t * CH + b * D : t * CH + (b + 1) * D],
                        rhs=w[:, c0 : c0 + QBLK],
                        start=(t == 0),
                        stop=(t == NKT - 1),
                        tile_position=(0, rb),
                        skip_group_check=True,
                    )

                return go

            # Ascending b: each b hits a distinct oacc region, so order is
            # free (one merged mult -> one wait sem either way); ascending
            # lets the per-j epilogue copies start while the last AVs of
            # higher batches still run, hiding the copy serialization.
            return [mk(b) for b in range(B)]

        # Software pipeline: the DVE backend for super-tile T is emitted
        # after the front end (scores+exp) of super-tile T+1, so every
        # cross-engine input reaching an engine's strict FIFO was produced
        # a super-tile earlier and waits are pre-satisfied. AV matmuls of T
        # are STAGED and only released into the drain queue during the front
        # end of T+3: releasing at T+2 put them at the PE queue head before
        # the producing mult finished, head-of-line-blocking the score
        # matmuls and stalling ScalarE ~1.5us per super-tile. The last
        # super-tile's backend runs per k-tile so its first half overlaps
        # the final exps (shorter kernel tail).
        e_tiles = {}
        av_stage = {}
        for T in range(NST + 1):
            if T - 3 in av_stage:
                av_pending.extend(av_stage.pop(T - 3))
            if T < NST:
                e_all = e_p.tile([128, 8192], f16, tag="e", name=f"e{T}")
                e_tiles[T] = e_all
                emit_scores_exp(T, 0, e_all)
                emit_scores_exp(T, 1, e_all)
            if T == 0:
                # Split the first backend per k-tile (its first half needs
                # only 4 exps -> DVE's pipeline starts ~4us earlier).
                w = w_p.tile([128, 8192], f16, tag="w", name="w0")
                av_stage[0] = backend_part(0, e_tiles[0], w, [0])
                av_stage[0] += backend_part(0, e_tiles.pop(0), w, [1])
            if T >= 2:
                X = T - 1
                w = w_p.tile([128, 8192], f16, tag="w", name=f"w{X}")
                if X == NST - 1:
                    # Split the last backend too: halves the mult->AV->copy
                    # latency chain in the kernel tail.
                    av_stage[X] = backend_part(X, e_tiles[X], w, [0])
                    av_stage[X] += backend_part(X, e_tiles.pop(X), w, [1])
                else:
                    av_stage[X] = backend_part(X, e_tiles.pop(X), w, [0, 1])
        for X in sorted(av_stage):
            av_pending.extend(av_stage.pop(X))
        drain_av(len(av_pending))

        # Epilogue: psum -> sbuf copies on ScalarE (idle in the tail, and
        # its PSUM read port is the fast one) as fp16 -- halves the final
        # DMA -- then one output DMA (4 separate dma_starts each pay ~2us
        # of setup/completion latency in the kernel tail).
        st = st_p.tile([128, NPAIR * QBLK], f16, tag="st")
        for j in range(NPAIR):
            nc.scalar.copy(out=st[:, j * QBLK : (j + 1) * QBLK], in_=oacc[j][:])
        nc.sync.dma_start(out=out_d[:], in_=st[:])

    return nc


def _get_nc():
    if "nc" not in _cache:
        nc = _build_nc()
        if not nc.is_finalized():
            # Runs Bacc.compile() legalization (wait splitting, reg alloc).
            nc.finalize()
        _cache["nc"] = nc
    return _cache["nc"]


def _host_prep(queries, keys, values):
    """Cast to fp16 and pre-arrange into the SBUF layouts (see _build_nc)."""
    k16 = np.asarray(keys, dtype=np.float16)
    v16 = np.asarray(values, dtype=np.float16)
    q16 = np.asarray(queries, dtype=np.float16)

    # kt[(b%2)*64+d, t*512 + (b//2)*128 + kk] = K[b, t*128+kk, d] (k-tile major)
    kt = np.ascontiguousarray(
        k16.reshape(NPAIR, 2, NKT, KT, D)
        .transpose(1, 4, 2, 0, 3)
        .reshape(128, NKT * NPAIR * KT)
    )
    # vv[p, t*512 + b*64 + d] = V[b, t*128+p, d] (k-tile major)
    vv = np.ascontiguousarray(
        v16.reshape(B, NKT, KT, D).transpose(2, 1, 0, 3).reshape(128, NKT * B * D)
    )

    qts = []
    for c in range(NCORES):
        qc = q16[:, c * QBLK : (c + 1) * QBLK, :]  # [8, 512, 64]
        qt = np.ascontiguousarray(
            qc.transpose(0, 2, 1).reshape(NPAIR, 128, QBLK).transpose(1, 0, 2).reshape(128, NPAIR * QBLK)
        )
        qts.append(qt)
    return qts, kt, vv


def kernel(queries, keys, values):
    global LAST_RESULT
    from concourse.bass_utils import run_bass_kernel_spmd

    queries = np.asarray(queries, dtype=np.float32)
    keys = np.asarray(keys, dtype=np.float32)
    values = np.asarray(values, dtype=np.float32)

    nc = _get_nc()
    qts, kt, vv = _host_prep(queries, keys, values)
    in_maps = [{"qt": qts[c], "kt": kt, "vv": vv} for c in range(NCORES)]

    res = run_bass_kernel_spmd(
        nc,
        in_maps,
        list(range(NCORES)),
        trace=TRACE,
        **TRACE_KWARGS,
    )
    LAST_RESULT = res

    out = np.empty((B, S, D), dtype=np.float32)
    for c in range(NCORES):
        o = res.results[c]["out"]  # [128, 2048] fp16 = [(b%2)*64+d, j*512+q]
        out[:, c * QBLK : (c + 1) * QBLK, :] = (
            o.astype(np.float32)
            .reshape(2, D, NPAIR, QBLK)
            .transpose(2, 0, 3, 1)
            .reshape(B, QBLK, D)
        )
    return out
